# revision 29
# baseline (speedup 1.0000x reference)
"""GRAM model Trainium2 kernel: 8-core SPMD via bass/tile.

Strategy (v2):
 - DAG embedding stage sharded over vocab (exact /8 shards): bf16 transposed
   gathers feed PE matmuls for h=tanh(cat@Wl.T+bl) (batched 512-wide tanh);
   per-group softmax weight sums reduced over partitions with tiny PE
   contractions; global sbar (13 floats) via AllGather + on-chip reduce.
 - Segment sum reformulated as a dense count-matrix matmul: each core holds
   its vocab shard of all_emb (bf16, built from normal-layout gathers scaled
   by sbar) and streams an fp8 count matrix C^T[v_shard, 4096 segs] (counts
   are small ints, exact in fp8) from HBM; partial le^T/re^T [H, 4096] are
   accumulated on PE and combined across cores by a ReduceScatter that lands
   each core's 512-segment block.
 - NTN head computed per core on its 512 graph pairs with batched matmuls.
Host side only does sharding prep: index remapping/padding and building the
count matrix from the integer node->graph assignments (the contiguous
graph-range sharding the hint asks for).
"""
import os
import numpy as np
import ml_dtypes

LAST_RESULT = None
LAST_EXEC_NS = None

H = 128
P16 = 16
B = 4096
T = 262144
V_D, V_P, V_A = 10000, 4000, 4000
LS = [4, 4, 5]
NCORE = 8
BLOC = B // NCORE          # 512 segments per core
VS = [1250, 500, 500]
VPAD = [1280, 512, 512]
NTIL = [10, 4, 4]
MOFF = [0, 10, 14]         # tile-column offsets into the mask array
GCOL = [0, 4, 8]           # sbar column offsets per group
GOFF_SH = [0, 1280, 1792]  # row offset of group inside a rank's shard
KOFF = [0, 10, 14]         # v-tile offset of group inside a rank's shard
SHROWS = 2304              # rows per rank shard (incl pads)
KT = SHROWS // 128         # 18 v-tiles per shard
EOFF = [0, 13000, 18200]   # group offsets in emb_cat (23400 rows)
NCH = 8                    # segment chunks of 512
CTW = KT * 512             # 9216 free-dim of a streamed C^T chunk


def _build_perm():
    perm = np.empty(18000, np.int64)
    v = np.arange(V_D)
    perm[:V_D] = (v // VS[0]) * SHROWS + GOFF_SH[0] + (v % VS[0])
    v = np.arange(V_P)
    perm[V_D:V_D + V_P] = (v // VS[1]) * SHROWS + GOFF_SH[1] + (v % VS[1])
    v = np.arange(V_A)
    perm[V_D + V_P:] = (v // VS[2]) * SHROWS + GOFF_SH[2] + (v % VS[2])
    return perm


def _wrap_idx(a):
    """dma_gather index layout: element i at [i%16, i//16]; replicate to 128 parts."""
    m = a.reshape(-1, 16).T.astype(np.int16)
    return np.ascontiguousarray(np.tile(m, (8, 1)))


def _build(inputs):
    import concourse.bacc as bacc
    import concourse.tile as tile
    import concourse.mybir as mybir
    from concourse import bass_isa

    f32 = mybir.dt.float32
    bf16 = mybir.dt.bfloat16
    fp16 = mybir.dt.float16
    i16 = mybir.dt.int16
    fp8 = mybir.dt.float8e4
    f32r = mybir.dt.float32r
    np_fp8 = mybir.dt.np(fp8)

    # ---------------- host-side shard prep ----------------
    lx = np.asarray(inputs["left_x"])[:, 0].astype(np.int64)
    rx = np.asarray(inputs["right_x"])[:, 0].astype(np.int64)
    lb = np.asarray(inputs["left_x_batch"]).astype(np.int64)
    rb = np.asarray(inputs["right_x_batch"]).astype(np.int64)

    perm = _build_perm()
    lut = np.arange(256).astype(np_fp8)

    def count_mat(pos, seg):
        cnt = np.bincount(pos * B + seg, minlength=SHROWS * NCORE * B)
        c8 = lut[np.minimum(cnt, 255).astype(np.uint8)]
        # [core, k, p, chunk, j] -> per-core [chunk, p, k*512+j]
        return c8.reshape(NCORE, KT, 128, NCH, 512)

    cml = count_mat(perm[lx], lb)
    cmr = count_mat(perm[rx], rb)

    def core_ct(c):
        al = cml[c].transpose(2, 1, 0, 3).reshape(NCH, 128, CTW)
        ar = cmr[c].transpose(2, 1, 0, 3).reshape(NCH, 128, CTW)
        return np.ascontiguousarray(np.stack([al, ar]))  # [2, 8, 128, CTW]

    anc = [np.asarray(inputs["anc_d"]), np.asarray(inputs["anc_p"]), np.asarray(inputs["anc_a"])]
    leaf = [np.asarray(inputs["leaf_d"]), np.asarray(inputs["leaf_p"]), np.asarray(inputs["leaf_a"])]
    DAGROWS = sum(VPAD[g] * LS[g] for g in range(3))   # 9728

    def dag_idx(tabs, core):
        out = np.zeros(DAGROWS, np.int64)
        off = 0
        for g in range(3):
            vsl = slice(core * VS[g], (core + 1) * VS[g])
            for l in range(LS[g]):
                out[off:off + VS[g]] = tabs[g][vsl, l] + EOFF[g]
                out[off + VS[g]:off + VPAD[g]] = EOFF[g]
                off += VPAD[g]
        return _wrap_idx(out)

    # per-partition validity mask, one column per v-tile of each group
    maskP = np.zeros((128, 18), np.float32)
    for g in range(3):
        for t in range(NTIL[g]):
            v0 = t * 128
            maskP[:, MOFF[g] + t] = (np.arange(v0, v0 + 128) < VS[g]).astype(np.float32)

    emb_cat = np.concatenate([np.asarray(inputs["emb_d"]),
                              np.asarray(inputs["emb_p"]),
                              np.asarray(inputs["emb_a"])], axis=0).astype(np.float32)
    emb16 = emb_cat.astype(ml_dtypes.bfloat16)
    emb32 = emb_cat
    wlA = np.concatenate([np.asarray(inputs[k])[:, :H].T for k in ("Wl_d", "Wl_p", "Wl_a")],
                         axis=1).astype(ml_dtypes.bfloat16)      # [128, 384]
    wlL = np.concatenate([np.asarray(inputs[k])[:, H:].T for k in ("Wl_d", "Wl_p", "Wl_a")],
                         axis=1).astype(ml_dtypes.bfloat16)
    bl3 = np.stack([np.asarray(inputs[k]) for k in ("bl_d", "bl_p", "bl_a")], axis=1).astype(np.float32)
    ap3 = np.concatenate([np.asarray(inputs[k]) for k in ("ap_d", "ap_p", "ap_a")], axis=1).astype(np.float32)
    W_ntn = np.asarray(inputs["W_ntn"]).astype(np.float32)
    wpk = np.concatenate([W_ntn[:, :, p] for p in range(P16)],
                         axis=1).astype(np.float32)              # [128, 2048]
    V_ntn = np.asarray(inputs["V_ntn"]).astype(np.float32)
    vlT = np.ascontiguousarray(V_ntn[:, :H].T)  # [128,16]
    vrT = np.ascontiguousarray(V_ntn[:, H:].T)
    bnt = np.asarray(inputs["b_ntn"]).astype(np.float32).reshape(P16, 1).copy()
    wfcbc = np.tile(np.asarray(inputs["w_fc"]).astype(np.float32).reshape(1, 1, P16),
                    (128, 4, 1)).astype(ml_dtypes.bfloat16).copy()     # [128,4,16]
    bfcbc = np.full((128, 1), float(np.asarray(inputs["b_fc"]).reshape(-1)[0]), np.float32)
    ones1 = np.ones((128, 1), np.float32)
    c16 = np.full((128, 1), 16.0, np.float32)
    I16 = np.eye(16, dtype=np.float32)

    shared = dict(emb16=emb16, emb32=emb32, wlA=wlA, wlL=wlL, bl3=bl3, ap3=ap3,
                  wpk=wpk, vlT=vlT, vrT=vrT, bnt=bnt, wfcbc=wfcbc, bfcbc=bfcbc,
                  ones1=ones1, c16=c16, I16=I16, maskP=maskP)
    in_maps = []
    for c in range(NCORE):
        m = dict(shared)
        m["aidx"] = dag_idx(anc, c)
        m["lidx"] = dag_idx(leaf, c)
        m["ct"] = core_ct(c)
        in_maps.append(m)

    # ---------------- device program ----------------
    nc = bacc.Bacc("TRN2", target_bir_lowering=False, debug=False,
                   enable_asserts=False, num_devices=NCORE)

    def din(name, arr, dt):
        return nc.dram_tensor(name, list(np.asarray(arr).shape), dt, kind="ExternalInput").ap()

    d_emb16 = din("emb16", emb16, bf16)
    d_emb32 = din("emb32", emb32, f32)
    d_wlA = din("wlA", wlA, bf16)
    d_wlL = din("wlL", wlL, bf16)
    d_bl3 = din("bl3", bl3, f32)
    d_ap3 = din("ap3", ap3, f32)
    d_wpk = din("wpk", wpk, f32)
    d_vlT = din("vlT", vlT, f32)
    d_vrT = din("vrT", vrT, f32)
    d_bnt = din("bnt", bnt, f32)
    d_wfcbc = din("wfcbc", wfcbc, bf16)
    d_bfcbc = din("bfcbc", bfcbc, f32)
    d_ones1 = din("ones1", ones1, f32)
    d_c16 = din("c16", c16, f32)
    d_I16 = din("I16", I16, f32)
    d_mask = din("maskP", maskP, f32)
    d_aidx = din("aidx", in_maps[0]["aidx"], i16)
    d_lidx = din("lidx", in_maps[0]["lidx"], i16)
    d_ct = din("ct", in_maps[0]["ct"], fp8)

    d_out = nc.dram_tensor("out", [1, BLOC], f32, kind="ExternalOutput").ap()

    d_sbin = nc.dram_tensor("sbin", [16], f32, kind="Internal").ap()
    d_sbga = nc.dram_tensor("sbga", [NCORE * 16], f32, kind="Internal", addr_space="Shared").ap()
    d_rsin = nc.dram_tensor("rsin", [2, NCH, 128, 512], f32, kind="Internal").ap()
    d_rsout = nc.dram_tensor("rsout", [2, 128, 512], f32, kind="Internal").ap()

    RG = [list(range(NCORE))]
    AT = mybir.ActivationFunctionType
    AL = mybir.AluOpType

    with tile.TileContext(nc) as tc:
        from contextlib import ExitStack
        est = ExitStack()
        with est:
            cpool = est.enter_context(tc.tile_pool(name="consts", bufs=1))
            dagp = est.enter_context(tc.tile_pool(name="dag", bufs=4))
            hpool = est.enter_context(tc.tile_pool(name="hsb", bufs=4))
            smp = est.enter_context(tc.tile_pool(name="smallsb", bufs=4))
            g32p = est.enter_context(tc.tile_pool(name="g32", bufs=2))
            accp = est.enter_context(tc.tile_pool(name="acc", bufs=4))
            ctp = est.enter_context(tc.tile_pool(name="ctsb", bufs=4))
            segp = est.enter_context(tc.tile_pool(name="segsb", bufs=2))
            lrp = est.enter_context(tc.tile_pool(name="lrsb", bufs=1))
            pcpp = est.enter_context(tc.tile_pool(name="pcpsb", bufs=1))
            hdp = est.enter_context(tc.tile_pool(name="headsb", bufs=4))

            _ldn = [0]
            def load(dram_ap, shape, dt):
                _ldn[0] += 1
                t = cpool.tile(shape, dt, tag=f"c{_ldn[0]}")
                nc.sync.dma_start(out=t[:], in_=dram_ap)
                return t

            t_wlA = load(d_wlA[:, :], [128, 384], bf16)
            t_wlL = load(d_wlL[:, :], [128, 384], bf16)
            t_bl3 = load(d_bl3[:, :], [128, 3], f32)
            t_ap3 = load(d_ap3[:, :], [128, 3], f32)
            t_wpk = load(d_wpk[:, :], [128, 2048], f32)
            t_vlT = load(d_vlT[:, :], [128, 16], f32)
            t_vrT = load(d_vrT[:, :], [128, 16], f32)
            t_bnt = load(d_bnt[:, :], [16, 1], f32)
            t_wfcbc = load(d_wfcbc[:, :, :], [128, 4, 16], bf16)
            t_bfcbc = load(d_bfcbc[:, :], [128, 1], f32)
            t_ones1 = load(d_ones1[:, :], [128, 1], f32)
            t_c16 = load(d_c16[:, :], [128, 1], f32)
            t_I16 = load(d_I16[:, :], [16, 16], f32)
            t_mask = load(d_mask[:, :], [128, 18], f32)
            t_aidx = load(d_aidx[:, :], [128, DAGROWS // 16], i16)
            t_lidx = load(d_lidx[:, :], [128, DAGROWS // 16], i16)

            # ---------- Phase A: attention logits + softmax partial sums ----------
            estA = ExitStack()
            ps_h = estA.enter_context(tc.tile_pool(name="psh", bufs=2, space="PSUM"))
            ps_aw = estA.enter_context(tc.tile_pool(name="psaw", bufs=2, space="PSUM"))
            ps_sb = estA.enter_context(tc.tile_pool(name="pssb", bufs=1, space="PSUM"))
            sacc_ps = ps_sb.tile([1, 16], f32)
            roff = 0
            for g in range(3):
                vp = VPAD[g]
                L = LS[g]
                nt = NTIL[g]
                awG = ps_aw.tile([128, 10, 16], f32, tag="aw")
                co = roff // 16
                nv = vp * L
                aT = dagp.tile([128, 1, 5120], bf16, tag="dag")
                nc.gpsimd.dma_gather(
                    out_ap=aT[:, :, :nv], in_ap=d_emb16[:, :],
                    idxs_ap=t_aidx[:, co:co + nv // 16],
                    num_idxs=nv, num_idxs_reg=nv, elem_size=H,
                    transpose=True, single_packet=False, queue_num=0)
                lT = dagp.tile([128, 1, 5120], bf16, tag="dag")
                nc.gpsimd.dma_gather(
                    out_ap=lT[:, :, :nv], in_ap=d_emb16[:, :],
                    idxs_ap=t_lidx[:, co:co + nv // 16],
                    num_idxs=nv, num_idxs_reg=nv, elem_size=H,
                    transpose=True, single_packet=False, queue_num=0)
                for l in range(L):
                    c0 = 0
                    while c0 < vp:
                        cw = min(512, vp - c0)
                        o0 = l * vp + c0
                        hp = ps_h.tile([128, 512], f32, tag="h")
                        nc.tensor.matmul(hp[:, :cw], t_wlA[:, g * 128:(g + 1) * 128],
                                         aT[:, 0, o0:o0 + cw], start=True, stop=False)
                        nc.tensor.matmul(hp[:, :cw], t_wlL[:, g * 128:(g + 1) * 128],
                                         lT[:, 0, o0:o0 + cw], start=False, stop=True)
                        hs = hpool.tile([128, 512], f32, tag="hs")
                        nc.scalar.activation(hs[:, :cw], hp[:, :cw], AT.Tanh,
                                             bias=t_bl3[:, g:g + 1])
                        for t0 in range(0, cw, 128):
                            t = (c0 + t0) // 128
                            nc.tensor.matmul(awG[:, t, l:l + 1],
                                             hs[:, t0:t0 + 128], t_ap3[:, g:g + 1],
                                             start=True, stop=True)
                        c0 += cw
                # softmax statistics: sbar_g[l] = sum_{v} exp(aw)/sum_l exp(aw)
                ex = smp.tile([128, 10, 16], f32, tag="ex")
                nc.scalar.activation(ex[:, :nt, :L], awG[:, :nt, :L], AT.Exp)
                den = smp.tile([128, 10, 1], f32, tag="den")
                nc.vector.tensor_reduce(den[:, :nt, :], ex[:, :nt, :L],
                                        axis=mybir.AxisListType.X, op=AL.add)
                rec = smp.tile([128, 10, 1], f32, tag="rec")
                nc.vector.reciprocal(rec[:, :nt, :], den[:, :nt, :])
                m2 = smp.tile([128, 10], f32, tag="m2")
                nc.vector.tensor_tensor(out=m2[:, :nt], in0=rec[:, :nt, 0],
                                        in1=t_mask[:, MOFF[g]:MOFF[g] + nt], op=AL.mult)
                for t in range(nt):
                    nc.tensor.matmul(sacc_ps[0:1, GCOL[g]:GCOL[g] + L],
                                     m2[:, t:t + 1], ex[:, t, :L],
                                     start=(t == 0), stop=(t == nt - 1))
                roff += vp * L

            # ---------- Phase B: global sbar (tiny AllGather) ----------
            sres = smp.tile([1, 16], f32, tag="sres")
            nc.vector.memset(sres[:], 0.0)
            nc.vector.tensor_copy(sres[:, 0:13], sacc_ps[0:1, 0:13])
            nc.sync.dma_start(out=d_sbin[:], in_=sres[0:1, :])
            estA.close()
            nc.gpsimd.collective_compute(
                "AllGather", AL.bypass, replica_groups=RG,
                ins=[d_sbin[:]], outs=[d_sbga[:]])
            t_sba = cpool.tile([8, 16], f32)
            nc.sync.dma_start(out=t_sba[:], in_=d_sbga[:])
            t_sbr = cpool.tile([8, 16], f32)
            nc.gpsimd.partition_all_reduce(t_sbr[:], t_sba[:], channels=8,
                                           reduce_op=bass_isa.ReduceOp.add)
            t_sbb0 = cpool.tile([128, 16], f32)
            nc.gpsimd.partition_broadcast(t_sbb0[:], t_sbr[0:1, :], channels=128)
            t_sbb = cpool.tile([128, 16], f32)
            nc.vector.tensor_scalar(out=t_sbb[:], in0=t_sbb0[:], scalar1=0.0625,
                                    scalar2=None, op0=AL.mult)

            # ---------- Phase C: all_emb/16 shard decomposed into 4 fp8 components ----------
            # split work across DVE and GpSimd (both idle here)
            acc_all = cpool.tile([128, KT, 128], f32)
            emb8 = []
            for i in range(4):
                e8c = cpool.tile([128, KT, 128], fp8, tag=f"e8{i}")
                emb8.append(e8c)
            roff = 0
            for g in range(3):
                vp = VPAD[g]
                L = LS[g]
                nt = NTIL[g]
                nv = vp * L
                gt = g32p.tile([128, 40, 128], f32, tag="g32")
                nc.gpsimd.dma_gather(
                    out_ap=gt[:, :nv // 128, :], in_ap=d_emb32[:, :],
                    idxs_ap=t_aidx[:, roff // 16:(roff + nv) // 16],
                    num_idxs=nv, num_idxs_reg=nv, elem_size=H,
                    transpose=False, single_packet=False, queue_num=0)
                for t in range(nt):
                    k = KOFF[g] + t
                    eng = nc.vector if t % 2 == 0 else nc.gpsimd
                    acc = acc_all[:, k, :]
                    eng.tensor_scalar(out=acc, in0=gt[:, t, :],
                                      scalar1=t_sbb[:, GCOL[g]:GCOL[g] + 1],
                                      scalar2=None, op0=AL.mult)
                    for l in range(1, L):
                        tmp = accp.tile([128, 128], f32, tag="tmp")
                        eng.tensor_scalar(out=tmp[:], in0=gt[:, l * nt + t, :],
                                          scalar1=t_sbb[:, GCOL[g] + l:GCOL[g] + l + 1],
                                          scalar2=None, op0=AL.mult)
                        eng.tensor_tensor(out=acc, in0=acc, in1=tmp[:],
                                          op=AL.add)
                    # fp8 decomposition: acc ~= e0 + e1 + (e2 + e3)/256
                    res = acc
                    for i in range(2):
                        eng.tensor_copy(emb8[i][:, k, :], res)
                        up = accp.tile([128, 128], f32, tag="up")
                        eng.tensor_copy(up[:], emb8[i][:, k, :])
                        nres = accp.tile([128, 128], f32, tag="nres")
                        eng.tensor_tensor(out=nres[:], in0=res, in1=up[:],
                                          op=AL.subtract)
                        res = nres[:]
                    for i in range(2, 4):
                        eng.tensor_scalar(out=emb8[i][:, k, :], in0=res,
                                          scalar1=256.0, scalar2=None, op0=AL.mult)
                        if i == 3:
                            break
                        up = accp.tile([128, 128], f32, tag="up")
                        eng.tensor_scalar(out=up[:], in0=emb8[i][:, k, :],
                                          scalar1=0.00390625, scalar2=None, op0=AL.mult)
                        nres = accp.tile([128, 128], f32, tag="nres")
                        eng.tensor_tensor(out=nres[:], in0=res, in1=up[:],
                                          op=AL.subtract)
                        res = nres[:]
                roff += vp * L

            # ---------- Phase M: partial le^T/re^T via count-matrix matmul ----------
            # left side first, then its ReduceScatter overlaps the right side
            ps_main = est.enter_context(tc.tile_pool(name="psm", bufs=2, space="PSUM"))
            for s in range(2):
                for c in range(NCH):
                    ct = ctp.tile([128, KT, 512], fp8, tag="ct")
                    nc.sync.dma_start(out=ct[:], in_=d_ct[s, c, :, :])
                    psA = ps_main.tile([128, 512], f32, tag="psA")
                    psB = ps_main.tile([128, 512], f32, tag="psB")
                    for i in range(4):
                        ps = psA if i < 2 else psB
                        for kp in range(KT // 2):
                            nc.tensor.matmul(
                                ps[:], emb8[i][:, 2 * kp:2 * kp + 2, :],
                                ct[:, 2 * kp:2 * kp + 2, :],
                                start=(i % 2 == 0 and kp == 0),
                                stop=(i % 2 == 1 and kp == KT // 2 - 1),
                                perf_mode=mybir.MatmulPerfMode.DoubleRow)
                    tmpB = segp.tile([128, 512], f32, tag="tmpB")
                    nc.scalar.activation(tmpB[:], psB[:], AT.Copy, scale=0.00390625)
                    sseg = segp.tile([128, 512], f32, tag="sseg")
                    nc.vector.tensor_tensor(out=sseg[:], in0=psA[:], in1=tmpB[:],
                                            op=AL.add)
                    nc.sync.dma_start(out=d_rsin[s, c, :, :], in_=sseg[:])
                nc.gpsimd.collective_compute(
                    "ReduceScatter", AL.add, replica_groups=RG,
                    ins=[d_rsin[s, :, :, :]], outs=[d_rsout[s, :, :]])
            leTs = lrp.tile([128, 512], f32, tag="leTs")
            nc.sync.dma_start(out=leTs[:], in_=d_rsout[0, :, :])
            reTs = lrp.tile([128, 512], f32, tag="reTs")
            nc.sync.dma_start(out=reTs[:], in_=d_rsout[1, :, :])
            leT = lrp.tile([128, 512], f32, tag="leT")
            nc.vector.tensor_scalar(out=leT[:], in0=leTs[:],
                                    scalar1=t_c16[:, 0:1], scalar2=None, op0=AL.mult)
            reT = lrp.tile([128, 512], f32, tag="reT")
            nc.vector.tensor_scalar(out=reT[:], in0=reTs[:],
                                    scalar1=t_c16[:, 0:1], scalar2=None, op0=AL.mult)

            # ---------- Phase F: NTN head ----------
            ps_hd = est.enter_context(tc.tile_pool(name="pshd", bufs=2, space="PSUM"))
            ps_bi = est.enter_context(tc.tile_pool(name="psbi", bufs=2, space="PSUM"))
            ps_bl = est.enter_context(tc.tile_pool(name="psbl", bufs=1, space="PSUM"))
            ps_pt = est.enter_context(tc.tile_pool(name="pspt", bufs=1, space="PSUM"))
            # block term: B[p, b] = V_l[p].le_b + V_r[p].re_b  (+ b_ntn)
            B_ps = ps_bl.tile([16, 512], f32)
            nc.tensor.matmul(B_ps[:], t_vlT[:, :], leT[:], start=True, stop=False)
            nc.tensor.matmul(B_ps[:], t_vrT[:, :], reT[:], start=False, stop=True)
            B_sb = hdp.tile([16, 512], f32, tag="Bsb")
            nc.vector.tensor_scalar(out=B_sb[:], in0=B_ps[:],
                                    scalar1=t_bnt[:, 0:1], scalar2=None, op0=AL.add)
            pairT = ps_pt.tile([128, 4, 16], f32, tag="pairT")
            for c4 in range(4):
                nc.tensor.matmul(pairT[:, c4, :], B_sb[:, c4 * 128:(c4 + 1) * 128],
                                 t_I16[:, :], start=True, stop=True)
            for p in range(P16):
                tp = ps_hd.tile([128, 512], f32, tag="tp")
                nc.tensor.matmul(tp[:], t_wpk[:, p * 128:(p + 1) * 128], leT[:],
                                 start=True, stop=True)
                ml = hdp.tile([128, 512], f32, tag="ml")
                nc.vector.tensor_tensor(out=ml[:], in0=tp[:], in1=reT[:], op=AL.mult)
                for c4 in range(4):
                    nc.tensor.matmul(pairT[:, c4, p:p + 1],
                                     ml[:, c4 * 128:(c4 + 1) * 128],
                                     t_ones1[:, 0:1], start=False, stop=True,
                                     skip_group_check=True)
            th = hdp.tile([128, 4, 16], bf16, tag="th")
            nc.scalar.activation(th[:], pairT[:], AT.Tanh)
            wm = hdp.tile([128, 4, 16], bf16, tag="wm")
            nc.vector.tensor_tensor(out=wm[:], in0=th[:], in1=t_wfcbc[:], op=AL.mult)
            rd = hdp.tile([128, 4], f32, tag="rd")
            nc.vector.tensor_reduce(rd[:], wm[:], axis=mybir.AxisListType.X, op=AL.add)
            sg = hdp.tile([128, 4], f32, tag="sg")
            nc.scalar.activation(sg[:], rd[:], AT.Sigmoid, bias=t_bfcbc[:, 0:1])
            for c4 in range(4):
                nc.sync.dma_start(out=d_out[0, c4 * 128:(c4 + 1) * 128], in_=sg[:, c4:c4 + 1])

    nc.compile()
    return nc, in_maps


def kernel(**inputs):
    from concourse.bass_utils import run_bass_kernel_spmd
    nc, in_maps = _build(inputs)
    res = run_bass_kernel_spmd(nc, in_maps, list(range(NCORE)))
    global LAST_RESULT, LAST_EXEC_NS
    LAST_RESULT = res
    if os.environ.get("KTIME"):
        import time as _time
        try:
            import jax
            from jax.sharding import Mesh, PartitionSpec, NamedSharding
            from jax.experimental.shard_map import shard_map
            import concourse.mybir as mybir2
            from concourse import bass2jax as b2j
            b2j.install_neuronx_cc_hook()
            in_names, out_names, out_avals, zero_outs = [], [], [], []
            pname = nc.partition_id_tensor.name if nc.partition_id_tensor else None
            for alloc in nc.m.functions[0].allocations:
                if not isinstance(alloc, mybir2.MemoryLocationSet):
                    continue
                name = alloc.memorylocations[0].name
                if alloc.kind == "ExternalInput":
                    if name != pname:
                        in_names.append(name)
                elif alloc.kind == "ExternalOutput":
                    shape = tuple(alloc.tensor_shape)
                    dtype = mybir2.dt.np(alloc.dtype)
                    out_names.append(name)
                    out_avals.append(jax.core.ShapedArray(shape, dtype))
                    zero_outs.append(np.zeros(shape, dtype))
            n_params = len(in_names)
            all_in = list(in_names) + list(out_names)
            if pname is not None:
                all_in.append(pname)

            def _body(*args):
                ops = list(args)
                if pname is not None:
                    ops.append(b2j.partition_id_tensor())
                return tuple(b2j._bass_exec_p.bind(
                    *ops, out_avals=tuple(out_avals), in_names=tuple(all_in),
                    out_names=tuple(out_names), lowering_input_output_aliases=(),
                    sim_require_finite=True, sim_require_nnan=True, nc=nc))

            devices = jax.devices()[:NCORE]
            mesh = Mesh(np.asarray(devices), ("core",))
            nio = n_params + len(out_names)
            fn = jax.jit(shard_map(_body, mesh=mesh,
                                   in_specs=(PartitionSpec("core"),) * nio,
                                   out_specs=(PartitionSpec("core"),) * len(out_names),
                                   check_rep=False),
                         donate_argnums=tuple(range(n_params, nio)), keep_unused=True)
            sh = NamedSharding(mesh, PartitionSpec("core"))
            conc = [jax.device_put(np.concatenate(
                        [np.asarray(in_maps[c][n]) for c in range(NCORE)], axis=0), sh)
                    for n in in_names]
            best = None
            for it in range(6):
                zs = [jax.device_put(np.zeros((NCORE * z.shape[0], *z.shape[1:]), z.dtype), sh)
                      for z in zero_outs]
                t0 = _time.perf_counter()
                out = fn(*conc, *zs)
                jax.block_until_ready(out)
                dt = _time.perf_counter() - t0
                if it > 0:
                    best = dt if best is None else min(best, dt)
            LAST_EXEC_NS = int(best * 1e9)
        except Exception as e:
            print("KTIME direct path failed:", repr(e))
    outs = [np.asarray(res.results[c]["out"]).reshape(BLOC) for c in range(NCORE)]
    return np.concatenate(outs).astype(np.float32)


if __name__ == "__main__":
    pass


# revision 37
# speedup vs baseline: 1.1566x; 1.1566x over previous
"""GRAM model Trainium2 kernel: 8-core SPMD via bass/tile.

Strategy (v2):
 - DAG embedding stage sharded over vocab (exact /8 shards): bf16 transposed
   gathers feed PE matmuls for h=tanh(cat@Wl.T+bl) (batched 512-wide tanh);
   per-group softmax weight sums reduced over partitions with tiny PE
   contractions; global sbar (13 floats) via AllGather + on-chip reduce.
 - Segment sum reformulated as a dense count-matrix matmul: each core holds
   its vocab shard of all_emb (bf16, built from normal-layout gathers scaled
   by sbar) and streams an fp8 count matrix C^T[v_shard, 4096 segs] (counts
   are small ints, exact in fp8) from HBM; partial le^T/re^T [H, 4096] are
   accumulated on PE and combined across cores by a ReduceScatter that lands
   each core's 512-segment block.
 - NTN head computed per core on its 512 graph pairs with batched matmuls.
Host side only does sharding prep: index remapping/padding and building the
count matrix from the integer node->graph assignments (the contiguous
graph-range sharding the hint asks for).
"""
import os
import numpy as np
import ml_dtypes

LAST_RESULT = None
LAST_EXEC_NS = None

H = 128
P16 = 16
B = 4096
T = 262144
V_D, V_P, V_A = 10000, 4000, 4000
LS = [4, 4, 5]
NCORE = 8
BLOC = B // NCORE          # 512 segments per core
VS = [1250, 500, 500]
VPAD = [1280, 512, 512]
NTIL = [10, 4, 4]
MOFF = [0, 10, 14]         # tile-column offsets into the mask array
GCOL = [0, 4, 8]           # sbar column offsets per group
GOFF_SH = [0, 1280, 1792]  # row offset of group inside a rank's shard
KOFF = [0, 10, 14]         # v-tile offset of group inside a rank's shard
SHROWS = 2304              # rows per rank shard (incl pads)
KT = SHROWS // 128         # 18 v-tiles per shard
EOFF = [0, 13000, 18200]   # group offsets in emb_cat (23400 rows)
NCH = 8                    # segment chunks of 512
CTW = KT * 512             # 9216 free-dim of a streamed C^T chunk


def _build_perm():
    perm = np.empty(18000, np.int64)
    v = np.arange(V_D)
    perm[:V_D] = (v // VS[0]) * SHROWS + GOFF_SH[0] + (v % VS[0])
    v = np.arange(V_P)
    perm[V_D:V_D + V_P] = (v // VS[1]) * SHROWS + GOFF_SH[1] + (v % VS[1])
    v = np.arange(V_A)
    perm[V_D + V_P:] = (v // VS[2]) * SHROWS + GOFF_SH[2] + (v % VS[2])
    return perm


def _wrap_idx(a):
    """dma_gather index layout: element i at [i%16, i//16]; replicate to 128 parts."""
    m = a.reshape(-1, 16).T.astype(np.int16)
    return np.ascontiguousarray(np.tile(m, (8, 1)))


def _build(inputs):
    import concourse.bacc as bacc
    import concourse.tile as tile
    import concourse.mybir as mybir
    from concourse import bass_isa

    f32 = mybir.dt.float32
    bf16 = mybir.dt.bfloat16
    fp16 = mybir.dt.float16
    i16 = mybir.dt.int16
    fp8 = mybir.dt.float8e4
    f32r = mybir.dt.float32r
    np_fp8 = mybir.dt.np(fp8)

    # ---------------- host-side shard prep ----------------
    lx = np.asarray(inputs["left_x"])[:, 0].astype(np.int64)
    rx = np.asarray(inputs["right_x"])[:, 0].astype(np.int64)
    lb = np.asarray(inputs["left_x_batch"]).astype(np.int64)
    rb = np.asarray(inputs["right_x_batch"]).astype(np.int64)

    perm = _build_perm()
    lut = np.arange(256).astype(np_fp8)

    def count_mat(pos, seg):
        cnt = np.bincount(pos * B + seg, minlength=SHROWS * NCORE * B)
        c8 = lut[np.minimum(cnt, 255).astype(np.uint8)]
        # [core, k, p, chunk, j] -> per-core [chunk, p, k*512+j]
        return c8.reshape(NCORE, KT, 128, NCH, 512)

    cml = count_mat(perm[lx], lb)
    cmr = count_mat(perm[rx], rb)

    def core_ct(c):
        al = cml[c].transpose(2, 1, 0, 3).reshape(NCH, 128, CTW)
        ar = cmr[c].transpose(2, 1, 0, 3).reshape(NCH, 128, CTW)
        return np.ascontiguousarray(np.stack([al, ar]))  # [2, 8, 128, CTW]

    anc = [np.asarray(inputs["anc_d"]), np.asarray(inputs["anc_p"]), np.asarray(inputs["anc_a"])]
    leaf = [np.asarray(inputs["leaf_d"]), np.asarray(inputs["leaf_p"]), np.asarray(inputs["leaf_a"])]
    DAGROWS = sum(VPAD[g] * LS[g] for g in range(3))   # 9728

    def dag_idx(tabs, core):
        out = np.zeros(DAGROWS, np.int64)
        off = 0
        for g in range(3):
            vsl = slice(core * VS[g], (core + 1) * VS[g])
            for l in range(LS[g]):
                out[off:off + VS[g]] = tabs[g][vsl, l] + EOFF[g]
                out[off + VS[g]:off + VPAD[g]] = EOFF[g]
                off += VPAD[g]
        return _wrap_idx(out)

    # per-partition validity mask, one column per v-tile of each group
    maskP = np.zeros((128, 18), np.float32)
    for g in range(3):
        for t in range(NTIL[g]):
            v0 = t * 128
            maskP[:, MOFF[g] + t] = (np.arange(v0, v0 + 128) < VS[g]).astype(np.float32)

    emb_cat = np.concatenate([np.asarray(inputs["emb_d"]),
                              np.asarray(inputs["emb_p"]),
                              np.asarray(inputs["emb_a"])], axis=0).astype(np.float32)
    emb16 = emb_cat.astype(ml_dtypes.bfloat16)
    emb32 = emb_cat
    wlA = np.concatenate([np.asarray(inputs[k])[:, :H].T for k in ("Wl_d", "Wl_p", "Wl_a")],
                         axis=1).astype(ml_dtypes.bfloat16)      # [128, 384]
    wlL = np.concatenate([np.asarray(inputs[k])[:, H:].T for k in ("Wl_d", "Wl_p", "Wl_a")],
                         axis=1).astype(ml_dtypes.bfloat16)
    bl3 = np.stack([np.asarray(inputs[k]) for k in ("bl_d", "bl_p", "bl_a")], axis=1).astype(np.float32)
    ap3 = np.concatenate([np.asarray(inputs[k]) for k in ("ap_d", "ap_p", "ap_a")], axis=1).astype(np.float32)
    W_ntn = np.asarray(inputs["W_ntn"]).astype(np.float32)
    wpk = np.concatenate([W_ntn[:, :, p] for p in range(P16)],
                         axis=1).astype(np.float32)              # [128, 2048]
    V_ntn = np.asarray(inputs["V_ntn"]).astype(np.float32)
    vlT = np.ascontiguousarray(V_ntn[:, :H].T)  # [128,16]
    vrT = np.ascontiguousarray(V_ntn[:, H:].T)
    bnt = np.asarray(inputs["b_ntn"]).astype(np.float32).reshape(P16, 1).copy()
    wfcbc = np.tile(np.asarray(inputs["w_fc"]).astype(np.float32).reshape(1, 1, P16),
                    (128, 4, 1)).astype(ml_dtypes.bfloat16).copy()     # [128,4,16]
    bfcbc = np.full((128, 1), float(np.asarray(inputs["b_fc"]).reshape(-1)[0]), np.float32)
    ones1 = np.ones((128, 1), np.float32)
    c16 = np.full((128, 1), 16.0, np.float32)
    I16 = np.eye(16, dtype=np.float32)

    shared = dict(emb16=emb16, emb32=emb32, wlA=wlA, wlL=wlL, bl3=bl3, ap3=ap3,
                  wpk=wpk, vlT=vlT, vrT=vrT, bnt=bnt, wfcbc=wfcbc, bfcbc=bfcbc,
                  ones1=ones1, c16=c16, I16=I16, maskP=maskP)
    in_maps = []
    for c in range(NCORE):
        m = dict(shared)
        m["aidx"] = dag_idx(anc, c)
        m["lidx"] = dag_idx(leaf, c)
        m["ct"] = core_ct(c)
        in_maps.append(m)

    # ---------------- device program ----------------
    nc = bacc.Bacc("TRN2", target_bir_lowering=False, debug=False,
                   enable_asserts=False, num_devices=NCORE)

    def din(name, arr, dt):
        return nc.dram_tensor(name, list(np.asarray(arr).shape), dt, kind="ExternalInput").ap()

    d_emb16 = din("emb16", emb16, bf16)
    d_emb32 = din("emb32", emb32, f32)
    d_wlA = din("wlA", wlA, bf16)
    d_wlL = din("wlL", wlL, bf16)
    d_bl3 = din("bl3", bl3, f32)
    d_ap3 = din("ap3", ap3, f32)
    d_wpk = din("wpk", wpk, f32)
    d_vlT = din("vlT", vlT, f32)
    d_vrT = din("vrT", vrT, f32)
    d_bnt = din("bnt", bnt, f32)
    d_wfcbc = din("wfcbc", wfcbc, bf16)
    d_bfcbc = din("bfcbc", bfcbc, f32)
    d_ones1 = din("ones1", ones1, f32)
    d_c16 = din("c16", c16, f32)
    d_I16 = din("I16", I16, f32)
    d_mask = din("maskP", maskP, f32)
    d_aidx = din("aidx", in_maps[0]["aidx"], i16)
    d_lidx = din("lidx", in_maps[0]["lidx"], i16)
    d_ct = din("ct", in_maps[0]["ct"], fp8)

    d_out = nc.dram_tensor("out", [1, BLOC], f32, kind="ExternalOutput").ap()

    d_sbin = nc.dram_tensor("sbin", [16], f32, kind="Internal").ap()
    d_sbga = nc.dram_tensor("sbga", [NCORE * 16], f32, kind="Internal", addr_space="Shared").ap()
    d_rsin = nc.dram_tensor("rsin", [2, NCH, 128, 512], f32, kind="Internal").ap()
    d_rsout = nc.dram_tensor("rsout", [2, 128, 512], f32, kind="Internal").ap()

    RG = [list(range(NCORE))]
    AT = mybir.ActivationFunctionType
    AL = mybir.AluOpType

    with tile.TileContext(nc) as tc:
        from contextlib import ExitStack
        est = ExitStack()
        with est:
            cpool = est.enter_context(tc.tile_pool(name="consts", bufs=1))
            dagp = est.enter_context(tc.tile_pool(name="dag", bufs=3))
            hpool = est.enter_context(tc.tile_pool(name="hsb", bufs=4))
            smp = est.enter_context(tc.tile_pool(name="smallsb", bufs=4))
            g32p = est.enter_context(tc.tile_pool(name="g32", bufs=2))
            accp = est.enter_context(tc.tile_pool(name="acc", bufs=2))
            ctp = est.enter_context(tc.tile_pool(name="ctsb", bufs=4))
            segp = est.enter_context(tc.tile_pool(name="segsb", bufs=2))
            lrp = est.enter_context(tc.tile_pool(name="lrsb", bufs=1))
            pcpp = est.enter_context(tc.tile_pool(name="pcpsb", bufs=1))
            hdp = est.enter_context(tc.tile_pool(name="headsb", bufs=4))

            _ldn = [0]
            def load(dram_ap, shape, dt):
                _ldn[0] += 1
                t = cpool.tile(shape, dt, tag=f"c{_ldn[0]}")
                nc.sync.dma_start(out=t[:], in_=dram_ap)
                return t

            t_aidx = load(d_aidx[:, :], [128, DAGROWS // 16], i16)
            t_lidx = load(d_lidx[:, :], [128, DAGROWS // 16], i16)
            t_wlA = load(d_wlA[:, :], [128, 384], bf16)
            t_wlL = load(d_wlL[:, :], [128, 384], bf16)
            t_bl3 = load(d_bl3[:, :], [128, 3], f32)
            t_ap3 = load(d_ap3[:, :], [128, 3], f32)
            t_mask = load(d_mask[:, :], [128, 18], f32)

            # ---------- Phase A: attention logits + softmax partial sums ----------
            estA = ExitStack()
            ps_h = estA.enter_context(tc.tile_pool(name="psh", bufs=2, space="PSUM"))
            ps_aw = estA.enter_context(tc.tile_pool(name="psaw", bufs=2, space="PSUM"))
            ps_sb = estA.enter_context(tc.tile_pool(name="pssb", bufs=1, space="PSUM"))
            sacc_ps = ps_sb.tile([1, 16], f32)
            roff = 0
            for g in range(3):
                vp = VPAD[g]
                L = LS[g]
                nt = NTIL[g]
                awG = ps_aw.tile([128, 10, 16], f32, tag="aw")
                co = roff // 16
                nv = vp * L
                aT = dagp.tile([128, 1, 5120], bf16, tag="dag")
                nc.gpsimd.dma_gather(
                    out_ap=aT[:, :, :nv], in_ap=d_emb16[:, :],
                    idxs_ap=t_aidx[:, co:co + nv // 16],
                    num_idxs=nv, num_idxs_reg=nv, elem_size=H,
                    transpose=True, single_packet=False, queue_num=0)
                lT = dagp.tile([128, 1, 5120], bf16, tag="dag")
                nc.gpsimd.dma_gather(
                    out_ap=lT[:, :, :nv], in_ap=d_emb16[:, :],
                    idxs_ap=t_lidx[:, co:co + nv // 16],
                    num_idxs=nv, num_idxs_reg=nv, elem_size=H,
                    transpose=True, single_packet=False, queue_num=0)
                for l in range(L):
                    c0 = 0
                    while c0 < vp:
                        cw = min(512, vp - c0)
                        o0 = l * vp + c0
                        hp = ps_h.tile([128, 512], f32, tag="h")
                        nc.tensor.matmul(hp[:, :cw], t_wlA[:, g * 128:(g + 1) * 128],
                                         aT[:, 0, o0:o0 + cw], start=True, stop=False)
                        nc.tensor.matmul(hp[:, :cw], t_wlL[:, g * 128:(g + 1) * 128],
                                         lT[:, 0, o0:o0 + cw], start=False, stop=True)
                        hs = hpool.tile([128, 512], f32, tag="hs")
                        nc.scalar.activation(hs[:, :cw], hp[:, :cw], AT.Tanh,
                                             bias=t_bl3[:, g:g + 1])
                        for t0 in range(0, cw, 128):
                            t = (c0 + t0) // 128
                            nc.tensor.matmul(awG[:, t, l:l + 1],
                                             hs[:, t0:t0 + 128], t_ap3[:, g:g + 1],
                                             start=True, stop=True)
                        c0 += cw
                # softmax statistics: sbar_g[l] = sum_{v} exp(aw)/sum_l exp(aw)
                ex = smp.tile([128, 10, 16], f32, tag="ex")
                nc.scalar.activation(ex[:, :nt, :L], awG[:, :nt, :L], AT.Exp)
                den = smp.tile([128, 10, 1], f32, tag="den")
                nc.vector.tensor_reduce(den[:, :nt, :], ex[:, :nt, :L],
                                        axis=mybir.AxisListType.X, op=AL.add)
                rec = smp.tile([128, 10, 1], f32, tag="rec")
                nc.vector.reciprocal(rec[:, :nt, :], den[:, :nt, :])
                m2 = smp.tile([128, 10], f32, tag="m2")
                nc.vector.tensor_tensor(out=m2[:, :nt], in0=rec[:, :nt, 0],
                                        in1=t_mask[:, MOFF[g]:MOFF[g] + nt], op=AL.mult)
                for t in range(nt):
                    nc.tensor.matmul(sacc_ps[0:1, GCOL[g]:GCOL[g] + L],
                                     m2[:, t:t + 1], ex[:, t, :L],
                                     start=(t == 0), stop=(t == nt - 1))
                roff += vp * L

            t_wpk = load(d_wpk[:, :], [128, 2048], f32)
            t_vlT = load(d_vlT[:, :], [128, 16], f32)
            t_vrT = load(d_vrT[:, :], [128, 16], f32)
            t_bnt = load(d_bnt[:, :], [16, 1], f32)
            t_wfcbc = load(d_wfcbc[:, :, :], [128, 4, 16], bf16)
            t_bfcbc = load(d_bfcbc[:, :], [128, 1], f32)
            t_ones1 = load(d_ones1[:, :], [128, 1], f32)
            t_c16 = load(d_c16[:, :], [128, 1], f32)
            t_I16 = load(d_I16[:, :], [16, 16], f32)

            # ---------- Phase B: global sbar (tiny AllGather) ----------
            sres = smp.tile([1, 16], f32, tag="sres")
            nc.vector.memset(sres[:], 0.0)
            nc.vector.tensor_copy(sres[:, 0:13], sacc_ps[0:1, 0:13])
            nc.sync.dma_start(out=d_sbin[:], in_=sres[0:1, :])
            estA.close()
            nc.gpsimd.collective_compute(
                "AllGather", AL.bypass, replica_groups=RG,
                ins=[d_sbin[:]], outs=[d_sbga[:]])
            t_sba = cpool.tile([8, 16], f32)
            nc.sync.dma_start(out=t_sba[:], in_=d_sbga[:])
            t_sbr = cpool.tile([8, 16], f32)
            nc.gpsimd.partition_all_reduce(t_sbr[:], t_sba[:], channels=8,
                                           reduce_op=bass_isa.ReduceOp.add)
            t_sbb0 = cpool.tile([128, 16], f32)
            nc.gpsimd.partition_broadcast(t_sbb0[:], t_sbr[0:1, :], channels=128)
            t_sbb = cpool.tile([128, 16], f32)
            nc.vector.tensor_scalar(out=t_sbb[:], in0=t_sbb0[:], scalar1=0.0625,
                                    scalar2=None, op0=AL.mult)

            # ---------- Phase C: all_emb/16 shard decomposed into 4 fp8 components ----------
            # acc = sum_l sbar_l/16 * anc_e_l  (fused multiply-add, batched per level)
            acc_all = cpool.tile([128, KT, 128], f32)
            emb8 = []
            for i in range(4):
                e8c = cpool.tile([128, KT, 128], fp8, tag=f"e8{i}")
                emb8.append(e8c)
            roff = 0
            for g in range(3):
                vp = VPAD[g]
                L = LS[g]
                nt = NTIL[g]
                nv = vp * L
                gt = g32p.tile([128, 40, 128], f32, tag="g32")
                nc.gpsimd.dma_gather(
                    out_ap=gt[:, :nv // 128, :], in_ap=d_emb32[:, :],
                    idxs_ap=t_aidx[:, roff // 16:(roff + nv) // 16],
                    num_idxs=nv, num_idxs_reg=nv, elem_size=H,
                    transpose=False, single_packet=False, queue_num=0)
                ka, kb = KOFF[g], KOFF[g] + nt
                if g == 0:
                    nc.vector.tensor_scalar(out=acc_all[:, ka:kb, :], in0=gt[:, :nt, :],
                                            scalar1=t_sbb[:, GCOL[g]:GCOL[g] + 1],
                                            scalar2=None, op0=AL.mult)
                    for l in range(1, L):
                        nc.vector.scalar_tensor_tensor(
                            out=acc_all[:, ka:kb, :], in0=gt[:, l * nt:(l + 1) * nt, :],
                            scalar=t_sbb[:, GCOL[g] + l:GCOL[g] + l + 1],
                            in1=acc_all[:, ka:kb, :], op0=AL.mult, op1=AL.add)
                else:
                    nc.gpsimd.tensor_scalar(out=acc_all[:, ka:kb, :], in0=gt[:, :nt, :],
                                            scalar1=t_sbb[:, GCOL[g]:GCOL[g] + 1],
                                            scalar2=None, op0=AL.mult)
                    for l in range(1, L):
                        tmp = accp.tile([128, 4, 128], f32, tag="tmpg")
                        nc.gpsimd.tensor_scalar(out=tmp[:], in0=gt[:, l * nt:(l + 1) * nt, :],
                                                scalar1=t_sbb[:, GCOL[g] + l:GCOL[g] + l + 1],
                                                scalar2=None, op0=AL.mult)
                        nc.gpsimd.tensor_tensor(out=acc_all[:, ka:kb, :],
                                                in0=acc_all[:, ka:kb, :],
                                                in1=tmp[:], op=AL.add)
                roff += vp * L
            # greedy fp8 decomposition in k-slices: acc ~= e0 + e1 + (e2 + e3)/256
            for k0 in range(0, KT, 6):
                sl = slice(k0, k0 + 6)
                a = acc_all[:, sl, :]
                nc.scalar.activation(emb8[0][:, sl, :], a, AT.Copy)
                r1 = accp.tile([128, 6, 128], f32, tag="r1")
                nc.vector.scalar_tensor_tensor(out=r1[:], in0=emb8[0][:, sl, :],
                                               scalar=-1.0, in1=a,
                                               op0=AL.mult, op1=AL.add)
                nc.scalar.activation(emb8[1][:, sl, :], r1[:], AT.Copy)
                r2 = accp.tile([128, 6, 128], f32, tag="r2")
                nc.vector.scalar_tensor_tensor(out=r2[:], in0=emb8[1][:, sl, :],
                                               scalar=-1.0, in1=r1[:],
                                               op0=AL.mult, op1=AL.add)
                nc.scalar.activation(emb8[2][:, sl, :], r2[:], AT.Copy, scale=256.0)
                r3 = accp.tile([128, 6, 128], f32, tag="r3")
                nc.vector.scalar_tensor_tensor(out=r3[:], in0=r2[:],
                                               scalar=256.0, in1=emb8[2][:, sl, :],
                                               op0=AL.mult, op1=AL.subtract)
                nc.scalar.activation(emb8[3][:, sl, :], r3[:], AT.Copy)

            # ---------- Phase M: partial le^T/re^T via count-matrix matmul ----------
            # left side first, then its ReduceScatter overlaps the right side
            estM = ExitStack()
            ps_main = estM.enter_context(tc.tile_pool(name="psm", bufs=2, space="PSUM"))
            for s in range(2):
                for c in range(NCH):
                    ct = ctp.tile([128, KT, 512], fp8, tag="ct")
                    nc.sync.dma_start(out=ct[:], in_=d_ct[s, c, :, :])
                    psA = ps_main.tile([128, 512], f32, tag="psA")
                    psB = ps_main.tile([128, 512], f32, tag="psB")
                    for i in range(4):
                        ps = psA if i < 2 else psB
                        for kp in range(KT // 2):
                            nc.tensor.matmul(
                                ps[:], emb8[i][:, 2 * kp:2 * kp + 2, :],
                                ct[:, 2 * kp:2 * kp + 2, :],
                                start=(i % 2 == 0 and kp == 0),
                                stop=(i % 2 == 1 and kp == KT // 2 - 1),
                                perf_mode=mybir.MatmulPerfMode.DoubleRow)
                    tmpB = segp.tile([128, 512], f32, tag="tmpB")
                    nc.scalar.activation(tmpB[:], psB[:], AT.Copy, scale=0.00390625)
                    sseg = segp.tile([128, 512], f32, tag="sseg")
                    nc.vector.tensor_tensor(out=sseg[:], in0=psA[:], in1=tmpB[:],
                                            op=AL.add)
                    nc.sync.dma_start(out=d_rsin[s, c, :, :], in_=sseg[:])
                nc.gpsimd.collective_compute(
                    "ReduceScatter", AL.add, replica_groups=RG,
                    ins=[d_rsin[s, :, :, :]], outs=[d_rsout[s, :, :]])
            estM.close()
            leTs = lrp.tile([128, 512], f32, tag="leTs")
            nc.sync.dma_start(out=leTs[:], in_=d_rsout[0, :, :])
            reTs = lrp.tile([128, 512], f32, tag="reTs")
            nc.sync.dma_start(out=reTs[:], in_=d_rsout[1, :, :])
            leT = lrp.tile([128, 512], f32, tag="leT")
            nc.vector.tensor_scalar(out=leT[:], in0=leTs[:],
                                    scalar1=t_c16[:, 0:1], scalar2=None, op0=AL.mult)
            reT = lrp.tile([128, 512], f32, tag="reT")
            nc.vector.tensor_scalar(out=reT[:], in0=reTs[:],
                                    scalar1=t_c16[:, 0:1], scalar2=None, op0=AL.mult)

            # ---------- Phase F: NTN head ----------
            ps_hd = est.enter_context(tc.tile_pool(name="pshd", bufs=6, space="PSUM"))
            ps_bi = est.enter_context(tc.tile_pool(name="psbi", bufs=2, space="PSUM"))
            ps_bl = est.enter_context(tc.tile_pool(name="psbl", bufs=1, space="PSUM"))
            ps_pt = est.enter_context(tc.tile_pool(name="pspt", bufs=1, space="PSUM"))
            # block term: B[p, b] = V_l[p].le_b + V_r[p].re_b  (+ b_ntn)
            pairT = ps_pt.tile([128, 4, 16], f32, tag="pairT")
            nc.vector.memset(pairT[:], 0.0)
            tps = []
            for p in range(P16):
                tp = ps_hd.tile([128, 512], f32, tag="tp")
                nc.tensor.matmul(tp[:], t_wpk[:, p * 128:(p + 1) * 128], leT[:],
                                 start=True, stop=True)
                tps.append(tp)
                ml = hdp.tile([128, 512], f32, tag="ml")
                nc.vector.tensor_tensor(out=ml[:], in0=tp[:], in1=reT[:], op=AL.mult)
                for c4 in range(4):
                    nc.tensor.matmul(pairT[:, c4, p:p + 1],
                                     ml[:, c4 * 128:(c4 + 1) * 128],
                                     t_ones1[:, 0:1],
                                     start=False, stop=True,
                                     skip_group_check=True)
            B_ps = ps_bl.tile([16, 512], f32)
            nc.tensor.matmul(B_ps[:], t_vlT[:, :], leT[:], start=True, stop=False)
            nc.tensor.matmul(B_ps[:], t_vrT[:, :], reT[:], start=False, stop=True)
            B_sb = hdp.tile([16, 512], f32, tag="Bsb")
            nc.vector.tensor_scalar(out=B_sb[:], in0=B_ps[:],
                                    scalar1=t_bnt[:, 0:1], scalar2=None, op0=AL.add)
            for c4 in range(4):
                nc.tensor.matmul(pairT[:, c4, :], B_sb[:, c4 * 128:(c4 + 1) * 128],
                                 t_I16[:, :], start=False, stop=True,
                                 skip_group_check=True)
            th = hdp.tile([128, 4, 16], bf16, tag="th")
            nc.scalar.activation(th[:], pairT[:], AT.Tanh)
            wm = hdp.tile([128, 4, 16], bf16, tag="wm")
            nc.vector.tensor_tensor(out=wm[:], in0=th[:], in1=t_wfcbc[:], op=AL.mult)
            rd = hdp.tile([128, 4], f32, tag="rd")
            nc.vector.tensor_reduce(rd[:], wm[:], axis=mybir.AxisListType.X, op=AL.add)
            sg = hdp.tile([128, 4], f32, tag="sg")
            nc.scalar.activation(sg[:], rd[:], AT.Sigmoid, bias=t_bfcbc[:, 0:1])
            for c4 in range(4):
                nc.sync.dma_start(out=d_out[0, c4 * 128:(c4 + 1) * 128], in_=sg[:, c4:c4 + 1])

    nc.compile()
    return nc, in_maps


def kernel(**inputs):
    from concourse.bass_utils import run_bass_kernel_spmd
    nc, in_maps = _build(inputs)
    res = run_bass_kernel_spmd(nc, in_maps, list(range(NCORE)))
    global LAST_RESULT, LAST_EXEC_NS
    LAST_RESULT = res
    if os.environ.get("KTIME"):
        import time as _time
        try:
            import jax
            from jax.sharding import Mesh, PartitionSpec, NamedSharding
            from jax.experimental.shard_map import shard_map
            import concourse.mybir as mybir2
            from concourse import bass2jax as b2j
            b2j.install_neuronx_cc_hook()
            in_names, out_names, out_avals, zero_outs = [], [], [], []
            pname = nc.partition_id_tensor.name if nc.partition_id_tensor else None
            for alloc in nc.m.functions[0].allocations:
                if not isinstance(alloc, mybir2.MemoryLocationSet):
                    continue
                name = alloc.memorylocations[0].name
                if alloc.kind == "ExternalInput":
                    if name != pname:
                        in_names.append(name)
                elif alloc.kind == "ExternalOutput":
                    shape = tuple(alloc.tensor_shape)
                    dtype = mybir2.dt.np(alloc.dtype)
                    out_names.append(name)
                    out_avals.append(jax.core.ShapedArray(shape, dtype))
                    zero_outs.append(np.zeros(shape, dtype))
            n_params = len(in_names)
            all_in = list(in_names) + list(out_names)
            if pname is not None:
                all_in.append(pname)

            def _body(*args):
                ops = list(args)
                if pname is not None:
                    ops.append(b2j.partition_id_tensor())
                return tuple(b2j._bass_exec_p.bind(
                    *ops, out_avals=tuple(out_avals), in_names=tuple(all_in),
                    out_names=tuple(out_names), lowering_input_output_aliases=(),
                    sim_require_finite=True, sim_require_nnan=True, nc=nc))

            devices = jax.devices()[:NCORE]
            mesh = Mesh(np.asarray(devices), ("core",))
            nio = n_params + len(out_names)
            fn = jax.jit(shard_map(_body, mesh=mesh,
                                   in_specs=(PartitionSpec("core"),) * nio,
                                   out_specs=(PartitionSpec("core"),) * len(out_names),
                                   check_rep=False),
                         donate_argnums=tuple(range(n_params, nio)), keep_unused=True)
            sh = NamedSharding(mesh, PartitionSpec("core"))
            conc = [jax.device_put(np.concatenate(
                        [np.asarray(in_maps[c][n]) for c in range(NCORE)], axis=0), sh)
                    for n in in_names]
            best = None
            for it in range(6):
                zs = [jax.device_put(np.zeros((NCORE * z.shape[0], *z.shape[1:]), z.dtype), sh)
                      for z in zero_outs]
                t0 = _time.perf_counter()
                out = fn(*conc, *zs)
                jax.block_until_ready(out)
                dt = _time.perf_counter() - t0
                if it > 0:
                    best = dt if best is None else min(best, dt)
            LAST_EXEC_NS = int(best * 1e9)
        except Exception as e:
            print("KTIME direct path failed:", repr(e))
    outs = [np.asarray(res.results[c]["out"]).reshape(BLOC) for c in range(NCORE)]
    return np.concatenate(outs).astype(np.float32)


if __name__ == "__main__":
    pass
